# Initial kernel scaffold
#
"""Trainium2 Bass kernel for nn_DecoderActor (graph-attention decoder head).

Math (per batch b of 16384, N=20 nodes, DM=128):
    xg   = max_n x[b,n,:]
    h    = x @ Wn.T + bn + (xg @ Wg.T + bg)
    K    = h @ Wk.T + bk ;  Q = h @ Wq.T + bq
    M    = K @ Q.T / sqrt(DM)
    p    = softmax_400(10*tanh(M) - 1e20*eye)

Host-side algebraic fusion (exact, fp64):
    K = x @ Ak.T + (Bk @ xg + ck),   Ak = Wk@Wn, Bk = Wk@Wg, ck = Wk@(bn+bg)+bk
    Q = x @ Aq.T + (Bq @ xg + cq)    (same with q)
so the device never materializes h: 2 dm x dm matmuls per token instead of 3,
plus 2 tiny per-batch matmuls.

Device pipeline (data-parallel over batch, 8 cores, fp16 compute / fp32 softmax):
  per super-tile of 24 batches (480 tokens), fully unrolled 86x:
    cast-DMA x fp32->fp16 -> PE transpose (4x 120 tokens) -> xT in SBUF
    DVE segmented reduce_max -> xg[128, 24]
    PE: K/Q = AkT/AqT @ xT (480-col fp16 matmuls), vk/vq = BkT/BqT @ xg
    DVE: K += vk (free-broadcast), Q += vq -> fp16 SBUF
    PE: 4x block matmul K_blk.T @ Q_blk -> M [120x120] (6 batches/block)
    ACT: tanh(M/sqrt(128)) full tile -> SBUF fp16
    DMA: gather 6 diagonal 20x20 blocks -> dense [120, 4*20]
    ACT exp(10*t), DVE mask diag, fp32 row-sums, PE block-sum matmul,
    DVE reciprocal + scale -> p, one strided DMA store.
"""

import os
import sys

import numpy as np

for _p in ("/opt/trn_rl_repo",):
    if _p not in sys.path and os.path.isdir(_p):
        sys.path.insert(0, _p)

N = 20
DM = 128
B = 16384
NCORES = 8
BPC = B // NCORES          # 2048 batches per core
ST_B = 24                  # batches per super-tile
BPAD = 2064                # padded per-core batches (86 super-tiles)
NST = BPAD // ST_B         # 86
TPS = ST_B * N             # 480 tokens per super-tile
NG = TPS // 120            # 4 transpose chunks of 120 tokens
NBLK = 120 // N            # 6 batches per M-block

# set by test.py to capture profiling info
TRACE = False
LAST_RESULTS = None


def _host_weights(Wg, bg, Wn, bn, Wk, bk, Wq, bq):
    W = [a.astype(np.float64) for a in (Wg, bg, Wn, bn, Wk, bk, Wq, bq)]
    Wg, bg, Wn, bn, Wk, bk, Wq, bq = W
    Ak = Wk @ Wn
    Aq = Wq @ Wn
    Bk = Wk @ Wg
    Bq = Wq @ Wg
    ck = Wk @ (bn + bg) + bk
    cq = Wq @ (bn + bg) + bq
    return {
        "akT": np.ascontiguousarray(Ak.T).astype(np.float16),
        "aqT": np.ascontiguousarray(Aq.T).astype(np.float16),
        "bkT": np.ascontiguousarray(Bk.T).astype(np.float16),
        "bqT": np.ascontiguousarray(Bq.T).astype(np.float16),
        "ck": ck.astype(np.float32).reshape(DM, 1),
        "cq": cq.astype(np.float32).reshape(DM, 1),
    }


def _build_program(wts):
    import concourse.bass as bass
    import concourse.mybir as mybir
    from concourse.tile import TileContext

    f16 = mybir.dt.float16
    f32 = mybir.dt.float32
    AF = mybir.ActivationFunctionType
    AX = mybir.AxisListType

    nc = bass.Bass()
    x_d = nc.dram_tensor("x", [BPAD * N, DM], f32, kind="ExternalInput")
    out_d = nc.dram_tensor("out", [BPAD * N * N], f32, kind="ExternalOutput")

    akT_d = nc.inline_tensor(wts["akT"], name="akT")
    aqT_d = nc.inline_tensor(wts["aqT"], name="aqT")
    bkT_d = nc.inline_tensor(wts["bkT"], name="bkT")
    bqT_d = nc.inline_tensor(wts["bqT"], name="bqT")
    ck_d = nc.inline_tensor(wts["ck"], name="ck")
    cq_d = nc.inline_tensor(wts["cq"], name="cq")
    ident_d = nc.inline_tensor(np.eye(120, dtype=np.float16), name="ident")
    # mask[p, m] = 0 where node-row (p % 20) == m (the softmax-excluded diagonal)
    msk = (1.0 - np.eye(N, dtype=np.float32))
    mask_np = np.tile(msk, (NBLK, 1)).astype(np.float16)         # [120, 20]
    mask_d = nc.inline_tensor(mask_np, name="mask")
    # block-sum indicator: o2[k, i] = 1 if k//20 == i//20
    blk = np.arange(120) // N
    o2_np = (blk[:, None] == blk[None, :]).astype(np.float32)    # [120, 120]
    o2_d = nc.inline_tensor(o2_np, name="o2")

    SC1 = float(1.0 / np.sqrt(DM))

    with TileContext(nc) as tc:
        with (
            tc.tile_pool(name="consts", bufs=1) as consts,
            tc.tile_pool(name="work", bufs=2) as work,
            tc.tile_pool(name="ps_xt", bufs=1, space="PSUM") as ps_xt,
            tc.tile_pool(name="ps_k", bufs=2, space="PSUM") as ps_k,
            tc.tile_pool(name="ps_q", bufs=2, space="PSUM") as ps_q,
            tc.tile_pool(name="ps_m", bufs=1, space="PSUM") as ps_m,
            tc.tile_pool(name="ps_v", bufs=1, space="PSUM") as ps_v,
            tc.tile_pool(name="ps_t", bufs=1, space="PSUM") as ps_t,
        ):
            akT_s = consts.tile([DM, DM], f16)
            nc.sync.dma_start(out=akT_s, in_=akT_d[:, :])
            aqT_s = consts.tile([DM, DM], f16)
            nc.sync.dma_start(out=aqT_s, in_=aqT_d[:, :])
            bkT_s = consts.tile([DM, DM], f16)
            nc.sync.dma_start(out=bkT_s, in_=bkT_d[:, :])
            bqT_s = consts.tile([DM, DM], f16)
            nc.sync.dma_start(out=bqT_s, in_=bqT_d[:, :])
            ck_s = consts.tile([DM, 1], f32)
            nc.sync.dma_start(out=ck_s, in_=ck_d[:, :])
            cq_s = consts.tile([DM, 1], f32)
            nc.sync.dma_start(out=cq_s, in_=cq_d[:, :])
            ident_s = consts.tile([120, 120], f16)
            nc.sync.dma_start(out=ident_s, in_=ident_d[:, :])
            mask_s = consts.tile([120, N], f16)
            nc.sync.dma_start(out=mask_s, in_=mask_d[:, :])
            o2_s = consts.tile([120, 120], f32)
            nc.sync.dma_start(out=o2_s, in_=o2_d[:, :])

            for st in range(NST):
                r0 = st * TPS  # first token row of this super-tile

                # ---- load + fp16 cast: x_sb[p, g, d] = x[r0 + g*120 + p, d]
                x_sb = work.tile([120, NG, DM], f16)
                src = bass.AP(x_d, r0 * DM, [[DM, 120], [120 * DM, NG], [1, DM]])
                nc.gpsimd.dma_start(out=x_sb[:, :, :], in_=src)

                # ---- transpose to xT[d, g, p] (token-major within chunk)
                xT_ps = ps_xt.tile([DM, NG, 120], f16)
                for g in range(NG):
                    nc.tensor.transpose(xT_ps[:, g, :], x_sb[:, g, :], ident_s[:, :])
                xT_sb = work.tile([DM, NG, 120], f16)
                nc.scalar.activation(xT_sb[:, :, :], xT_ps[:, :, :], AF.Copy)

                # ---- xg[d, (g,c)] = max over the 20 nodes of each batch
                xg_sb = work.tile([DM, ST_B], f16)
                nc.vector.reduce_max(
                    out=xg_sb[:, :],
                    in_=xT_sb[:, :, :].rearrange("d g (c n) -> d g c n", n=N),
                    axis=AX.X,
                )

                # ---- per-batch bias vectors vk/vq = B @ xg + c
                v_ps = ps_v.tile([DM, 2, ST_B], f32)
                nc.tensor.matmul(v_ps[:, 0, :], bkT_s[:, :], xg_sb[:, :])
                nc.tensor.matmul(v_ps[:, 1, :], bqT_s[:, :], xg_sb[:, :])
                v_sb = work.tile([DM, 2, ST_B], f32)
                nc.scalar.activation(v_sb[:, 0, :], v_ps[:, 0, :], AF.Copy, bias=ck_s[:, 0:1])
                nc.scalar.activation(v_sb[:, 1, :], v_ps[:, 1, :], AF.Copy, bias=cq_s[:, 0:1])

                # ---- K/Q = A @ xT  (+ per-batch v, broadcast over the 20 nodes)
                k_ps = ps_k.tile([DM, TPS], f32)
                nc.tensor.matmul(k_ps[:, :], akT_s[:, :], xT_sb[:, :, :])
                q_ps = ps_q.tile([DM, TPS], f32)
                nc.tensor.matmul(q_ps[:, :], aqT_s[:, :], xT_sb[:, :, :])

                kq_sb = work.tile([DM, 2, NG, 120], f16)
                for i, src_ps in enumerate((k_ps, q_ps)):
                    vsl = v_sb[:, i, :]
                    vb = bass.AP(
                        vsl.tensor,
                        vsl.offset,
                        [vsl.ap[0], [NBLK, NG], [1, NBLK], [0, N]],
                    )
                    nc.vector.tensor_add(
                        kq_sb[:, i, :, :].rearrange("d g (c n) -> d g c n", n=N),
                        src_ps[:, :].rearrange("d (g c n) -> d g c n", g=NG, n=N),
                        vb,
                    )

                # ---- M blocks: [120 x 120] = K_blk.T @ Q_blk (6 batches each)
                m_ps = ps_m.tile([120, NG, 120], f32)
                for j in range(NG):
                    nc.tensor.matmul(
                        m_ps[:, j, :], kq_sb[:, 0, j, :], kq_sb[:, 1, j, :]
                    )

                # ---- tanh(M / sqrt(dm)) on the full tile (ACT reads PSUM)
                t_sb = work.tile([120, NG, 120], f16)
                nc.scalar.activation(t_sb[:, :, :], m_ps[:, :, :], AF.Tanh, scale=SC1)

                # ---- gather the 6 diagonal 20x20 blocks -> dense [120, (g, m)]
                d_sb = work.tile([120, NG, N], f16)
                for c in range(NBLK):
                    nc.sync.dma_start(
                        out=d_sb[c * N : (c + 1) * N, :, :],
                        in_=t_sb[c * N : (c + 1) * N, :, c * N : (c + 1) * N],
                    )

                # ---- exp(10 * t), mask diagonal
                e_sb = work.tile([120, NG, N], f16)
                nc.scalar.activation(e_sb[:, :, :], d_sb[:, :, :], AF.Exp, scale=10.0)
                em_sb = work.tile([120, NG, N], f16)
                mb = bass.AP(mask_s.tensor, mask_s.offset, [mask_s.ap[0], [0, NG], [1, N]])
                nc.vector.tensor_mul(em_sb[:, :, :], e_sb[:, :, :], mb)

                # ---- softmax denominators: row-sums then per-block totals
                r_sb = work.tile([120, NG], f32)
                nc.vector.reduce_sum(out=r_sb[:, :], in_=em_sb[:, :, :], axis=AX.X)
                tot_ps = ps_t.tile([120, NG], f32)
                nc.tensor.matmul(tot_ps[:, :], o2_s[:, :], r_sb[:, :])
                trec_sb = work.tile([120, NG], f32)
                nc.vector.reciprocal(trec_sb[:, :], tot_ps[:, :])

                # ---- p = e * (1/total), fp32 out
                p_sb = work.tile([120, NG, N], f32)
                tb = bass.AP(trec_sb.tensor, trec_sb.offset, [trec_sb.ap[0], [1, NG], [0, N]])
                nc.vector.tensor_mul(p_sb[:, :, :], em_sb[:, :, :], tb)

                # ---- store: out[b0 + j*6 + c, n, m] with p = 20c + n
                dst = bass.AP(
                    out_d, st * ST_B * N * N, [[N, 120], [NBLK * N * N, NG], [1, N]]
                )
                nc.sync.dma_start(out=dst, in_=p_sb[:, :, :])

    return nc


def kernel(x, Wg, bg, Wn, bn, Wk, bk, Wq, bq):
    global LAST_RESULTS
    from concourse.bass_utils import run_bass_kernel_spmd

    wts = _host_weights(Wg, bg, Wn, bn, Wk, bk, Wq, bq)
    nc = _build_program(wts)

    x = np.ascontiguousarray(x, dtype=np.float32)
    in_maps = []
    for i in range(NCORES):
        xi = np.zeros((BPAD * N, DM), dtype=np.float32)
        xi[: BPC * N] = x[i * BPC : (i + 1) * BPC].reshape(BPC * N, DM)
        in_maps.append({"x": xi})

    kwargs = {}
    if TRACE:
        kwargs = dict(trace=True, trace_cores=[0])
    res = run_bass_kernel_spmd(nc, in_maps, core_ids=list(range(NCORES)), **kwargs)
    LAST_RESULTS = res

    out = np.empty((B, N, N), dtype=np.float32)
    for i in range(NCORES):
        out[i * BPC : (i + 1) * BPC] = res.results[i]["out"].reshape(BPAD, N, N)[:BPC]
    return out


# revision 27
# speedup vs baseline: 41.7768x; 41.7768x over previous
"""Trainium2 Bass kernel for nn_DecoderActor (graph-attention decoder head).

Math (per batch b of 16384, N=20 nodes, DM=128):
    xg   = max_n x[b,n,:]
    h    = x @ Wn.T + bn + (xg @ Wg.T + bg)
    K    = h @ Wk.T + bk ;  Q = h @ Wq.T + bq
    M    = K @ Q.T / sqrt(DM)
    p    = softmax_400(10*tanh(M) - 1e20*eye)

Host-side algebraic fusion (exact, fp64):
    K = x @ Ak.T + (Bk @ xg + ck),   Ak = Wk@Wn, Bk = Wk@Wg, ck = Wk@(bn+bg)+bk
    Q = x @ Aq.T + (Bq @ xg + cq)    (same with q)
so the device never materializes h: per token one Ak and one Aq matmul, with
the per-batch affine part folded into the same PSUM accumulation.

Device pipeline (data-parallel over batch, 8 cores, fp16 compute / fp32
softmax).  Per super-tile of 48 batches (960 tokens), fully unrolled 43x:
  - SWDGE cast-DMA x fp32->fp16 (one strided load)
  - PE transpose (8 chunks of 120 tokens) -> xT fp16 in PSUM
  - ACT copies xT PSUM->SBUF; DVE segmented reduce_max -> xg[128, 48]
  - K/Q accumulated in PSUM as  ck (x) ones  (+)  Bk @ xg-bcast  (+)  Ak @ xT
    (two 480-column halves each; one PSUM bank per half)
  - fp16 copies: K on DVE, Q on ACT
  - PE: 48 small matmuls K_b.T @ Q_b -> [20x20], written DENSE into one PSUM
    tile [128, 12, 20] via col-group tile_position; batch (g, c) lands at
    partition group g%4, free slot (g//4)*6+c.
  - ACT tanh(M/sqrt(dm)) -> fp16, ACT exp(10*t), DVE diag/garbage mask,
    fp32 row-sums, PE block-sum matmul, DVE reciprocal + scale
  - 4 strided DMA stores (one per partition group), alternating HWDGE queues.
"""

import os
import sys

import numpy as np

for _p in ("/opt/trn_rl_repo",):
    if _p not in sys.path and os.path.isdir(_p):
        sys.path.insert(0, _p)

N = 20
DM = 128
B = 16384
NCORES = 8
BPC = B // NCORES          # 2048 batches per core
ST_B = 48                  # batches per super-tile
BPAD = 2064                # padded per-core batches (43 super-tiles of 48)
NST = BPAD // ST_B         # 43
TPS = ST_B * N             # 960 tokens per super-tile
NG = TPS // 120            # 8 chunks of 120 tokens
NBC = ST_B // 4            # 12 batch slots per PE column group
NBLK = 6                   # batches per chunk

TRACE = False              # unused; kept for compatibility
LAST_RESULTS = None


def _host_weights(Wg, bg, Wn, bn, Wk, bk, Wq, bq):
    W = [np.asarray(a).astype(np.float64) for a in (Wg, bg, Wn, bn, Wk, bk, Wq, bq)]
    Wg, bg, Wn, bn, Wk, bk, Wq, bq = W
    Ak = Wk @ Wn
    Aq = Wq @ Wn
    Bk = Wk @ Wg
    Bq = Wq @ Wg
    ck = Wk @ (bn + bg) + bk
    cq = Wq @ (bn + bg) + bq
    return {
        "akT": np.ascontiguousarray(Ak.T).astype(np.float16),
        "aqT": np.ascontiguousarray(Aq.T).astype(np.float16),
        "bkT": np.ascontiguousarray(Bk.T).astype(np.float16),
        "bqT": np.ascontiguousarray(Bq.T).astype(np.float16),
        "ckT": ck.astype(np.float16).reshape(1, DM),
        "cqT": cq.astype(np.float16).reshape(1, DM),
    }


def _build_program(wts, bench_x=None, reps=1):
    import concourse.bacc as bacc
    import concourse.bass as bass
    import concourse.mybir as mybir
    from concourse.tile import TileContext

    f16 = mybir.dt.float16
    f32 = mybir.dt.float32
    AF = mybir.ActivationFunctionType
    AX = mybir.AxisListType

    nc = bacc.Bacc()
    if bench_x is not None:
        # bench variant: x baked into the NEFF, result kept on-device, so a
        # run only ships the 4-byte tick/tock pair (isolates HW exec time)
        x_d = nc.inline_tensor(bench_x, name="xconst")
        out_d = nc.dram_tensor("out", [BPAD * N * N], f32)
    else:
        x_d = nc.dram_tensor("x", [BPAD * N, DM], f32, kind="ExternalInput")
        out_d = nc.dram_tensor("out", [BPAD * N * N], f32, kind="ExternalOutput")
    tick_d = nc.dram_tensor("tick", [1, 1], f32, kind="ExternalInput")
    tock_d = nc.dram_tensor("tock", [1, 1], f32, kind="ExternalOutput")

    akT_d = nc.inline_tensor(wts["akT"], name="akT")
    aqT_d = nc.inline_tensor(wts["aqT"], name="aqT")
    bkT_d = nc.inline_tensor(wts["bkT"], name="bkT")
    bqT_d = nc.inline_tensor(wts["bqT"], name="bqT")
    ckT_d = nc.inline_tensor(wts["ckT"], name="ckT")
    cqT_d = nc.inline_tensor(wts["cqT"], name="cqT")
    ones_d = nc.inline_tensor(np.ones((1, 1), dtype=np.float16), name="ones")
    ident_d = nc.inline_tensor(np.eye(120, dtype=np.float16), name="ident")

    # dense-M tile: batch (g, c) sits at partition group jj = g%4 (partitions
    # [32jj, 32jj+20)), free slot i = (g//4)*6 + c.
    # mask[p, m] = 1 only for valid node rows (p%32 < 20) off the diagonal.
    pp = np.arange(DM)
    nn_ = pp % 32
    valid = nn_ < N
    mask_np = (valid[:, None] & (nn_[:, None] != np.arange(N)[None, :])).astype(
        np.float16
    )  # [128, 20]
    mask_d = nc.inline_tensor(mask_np, name="mask")
    # block-sum indicator: o2[k, p] = 1 iff same partition group and both valid
    o2_np = (
        (pp[:, None] // 32 == pp[None, :] // 32) & valid[:, None] & valid[None, :]
    ).astype(np.float32)  # [128, 128]
    o2_d = nc.inline_tensor(o2_np, name="o2")

    SC1 = float(1.0 / np.sqrt(DM))
    H = TPS // 2  # 480-column halves for the K/Q matmuls

    with TileContext(nc) as tc:
        with (
            tc.tile_pool(name="consts", bufs=1) as consts,
            tc.tile_pool(name="work", bufs=3) as work,
            # p_sb is read by the output DMAs; giving every super-tile its own
            # buffer keeps store-completion waits off the compute engines
            tc.tile_pool(name="pout", bufs=NST) as pout,
            tc.tile_pool(name="mres", bufs=1, space="PSUM") as mres,
            tc.tile_pool(name="ps_xt", bufs=1, space="PSUM") as ps_xt,
            tc.tile_pool(name="ps_k", bufs=1, space="PSUM") as ps_k,
            tc.tile_pool(name="ps_q", bufs=1, space="PSUM") as ps_q,
            tc.tile_pool(name="ps_t", bufs=1, space="PSUM") as ps_t,
        ):
            nc.sync.dma_start(out=tock_d[:, :], in_=tick_d[:, :])

            akT_s = consts.tile([DM, DM], f16)
            nc.sync.dma_start(out=akT_s, in_=akT_d[:, :])
            aqT_s = consts.tile([DM, DM], f16)
            nc.sync.dma_start(out=aqT_s, in_=aqT_d[:, :])
            bkT_s = consts.tile([DM, DM], f16)
            nc.sync.dma_start(out=bkT_s, in_=bkT_d[:, :])
            bqT_s = consts.tile([DM, DM], f16)
            nc.sync.dma_start(out=bqT_s, in_=bqT_d[:, :])
            ckT_s = consts.tile([1, DM], f16)
            nc.sync.dma_start(out=ckT_s, in_=ckT_d[:, :])
            cqT_s = consts.tile([1, DM], f16)
            nc.sync.dma_start(out=cqT_s, in_=cqT_d[:, :])
            ones_s = consts.tile([1, 1], f16)
            nc.sync.dma_start(out=ones_s, in_=ones_d[:, :])
            ident_s = consts.tile([120, 120], f16)
            nc.sync.dma_start(out=ident_s, in_=ident_d[:, :])
            mask_s = consts.tile([DM, N], f16)
            nc.sync.dma_start(out=mask_s, in_=mask_d[:, :])
            o2_s = consts.tile([DM, DM], f32)
            nc.sync.dma_start(out=o2_s, in_=o2_d[:, :])

            # two manually-rotated dense-M PSUM tiles; memset once so the
            # never-written partition rows (p%32 >= 20) can't hold NaNs.
            m_ps0 = mres.tile([DM, NBC, N], f32)
            m_ps1 = mres.tile([DM, NBC, N], f32)
            nc.vector.memset(m_ps0[:, :, :], 0.0)
            nc.vector.memset(m_ps1[:, :, :], 0.0)

            for rep_st in range(reps * NST):
                st = rep_st % NST
                r0 = st * TPS  # first token row of this super-tile

                tot_ps = ps_t.tile([DM, NBC], f32, tag="tot")

                # ---- load + f16 cast: x_sb[p, g, d] = x[r0 + g*120 + p, d]
                x_sb = work.tile([120, NG, DM], f16)
                src = bass.AP(x_d, r0 * DM, [[DM, 120], [120 * DM, NG], [1, DM]])
                nc.gpsimd.dma_start(out=x_sb[:, :, :], in_=src)

                # ---- transpose to xT[d, (g, p)]
                xT_ps = ps_xt.tile([DM, NG, 120], f16)
                for g in range(NG):
                    nc.tensor.transpose(xT_ps[:, g, :], x_sb[:, g, :], ident_s[:, :])
                xT_sb = work.tile([DM, NG, 120], f16)
                nc.scalar.activation(xT_sb[:, :, :], xT_ps[:, :, :], AF.Copy)

                # ---- xg[d, (g, c)] = max over the 20 nodes of each batch
                # (read straight from PSUM so it doesn't wait on the ACT copy)
                xg_sb = work.tile([DM, ST_B], f16)
                nc.vector.reduce_max(
                    out=xg_sb[:, :],
                    in_=xT_ps[:, :, :].rearrange("d g (c n) -> d g c n", n=N),
                    axis=AX.X,
                )

                # broadcast APs for the bias accumulation matmuls
                ones_bc = bass.AP(ones_s.tensor, ones_s.offset, [ones_s.ap[0], [0, H]])

                def xg_bc(h):
                    return bass.AP(
                        xg_sb.tensor,
                        xg_sb.offset + h * (ST_B // 2),
                        [xg_sb.ap[0], [NBLK, NG // 2], [1, NBLK], [0, N]],
                    )

                # ---- K/Q = c (+) B @ xg (+) A @ xT, accumulated in PSUM
                # (two 480-column halves; each half is one PSUM bank)
                kq_ps = []
                for nm_, cT, bT, aT, pool in (
                    ("k_ps", ckT_s, bkT_s, akT_s, ps_k),
                    ("q_ps", cqT_s, bqT_s, aqT_s, ps_q),
                ):
                    ph = pool.tile([DM, 2, H], f32, name=nm_)
                    for h in range(2):
                        nc.tensor.matmul(
                            ph[:, h, :], cT[:, :], ones_bc, start=True, stop=False
                        )
                        nc.tensor.matmul(
                            ph[:, h, :], bT[:, :], xg_bc(h), start=False, stop=False
                        )
                        nc.tensor.matmul(
                            ph[:, h, :],
                            aT[:, :],
                            xT_sb[:, h * 4 : h * 4 + 4, :],
                            start=False,
                            stop=True,
                        )
                    kq_ps.append(ph)

                # ---- fp16 copies for the M matmuls (K on DVE, Q on ACT)
                kq_sb = work.tile([DM, 2, NG, 120], f16)
                nc.vector.tensor_copy(kq_sb[:, 0, :, :], kq_ps[0][:, :, :])
                nc.scalar.activation(kq_sb[:, 1, :, :], kq_ps[1][:, :, :], AF.Copy)

                # ---- dense M: batch (g, c) -> partition group g%4, free slot
                m_ps = m_ps0 if st % 2 == 0 else m_ps1
                for c in range(NBLK):
                    for g in range(NG):
                        jj = g % 4
                        i = (g // 4) * NBLK + c
                        nc.tensor.matmul(
                            m_ps[32 * jj : 32 * jj + N, i, :],
                            kq_sb[:, 0, g, c * N : (c + 1) * N],
                            kq_sb[:, 1, g, c * N : (c + 1) * N],
                            tile_position=(0, 32 * jj),
                        )

                # ---- softmax over the 400 entries of each batch
                t_sb = work.tile([DM, NBC, N], f16)
                nc.scalar.activation(t_sb[:, :, :], m_ps[:, :, :], AF.Tanh, scale=SC1)
                e_sb = work.tile([DM, NBC, N], f16)
                nc.scalar.activation(e_sb[:, :, :], t_sb[:, :, :], AF.Exp, scale=10.0)
                em_sb = work.tile([DM, NBC, N], f16)
                mb = bass.AP(
                    mask_s.tensor, mask_s.offset, [mask_s.ap[0], [0, NBC], [1, N]]
                )
                nc.vector.tensor_mul(em_sb[:, :, :], e_sb[:, :, :], mb)

                r_sb = work.tile([DM, NBC], f32)
                nc.vector.reduce_sum(out=r_sb[:, :], in_=em_sb[:, :, :], axis=AX.X)
                nc.tensor.matmul(tot_ps[:, :], o2_s[:, :], r_sb[:, :])
                trec_sb = work.tile([DM, NBC], f32)
                nc.vector.reciprocal(trec_sb[:, :], tot_ps[:, :])

                p_sb = pout.tile([DM, NBC, N], f32)
                tb = bass.AP(
                    trec_sb.tensor, trec_sb.offset, [trec_sb.ap[0], [1, NBC], [0, N]]
                )
                nc.vector.tensor_mul(p_sb[:, :, :], em_sb[:, :, :], tb)

                # ---- store: batch b0 + 24*(i//6) + 6*jj + (i%6) at partition
                # 32*jj + n, free (i, m).  One DMA per partition group.
                for jj in range(4):
                    dst = bass.AP(
                        out_d,
                        (st * ST_B + jj * NBLK) * N * N,
                        [[N, N], [24 * N * N, 2], [N * N, NBLK], [1, N]],
                    )
                    eng = nc.sync if jj % 2 == 0 else nc.scalar
                    eng.dma_start(out=dst, in_=p_sb[32 * jj : 32 * jj + N, :, :])

    nc.compile()
    return nc


def kernel(x, Wg, bg, Wn, bn, Wk, bk, Wq, bq):
    global LAST_RESULTS
    from concourse.bass_utils import run_bass_kernel_spmd

    wts = _host_weights(Wg, bg, Wn, bn, Wk, bk, Wq, bq)
    nc = _build_program(wts)

    x = np.ascontiguousarray(x, dtype=np.float32)
    tick = np.zeros((1, 1), dtype=np.float32)
    in_maps = []
    for i in range(NCORES):
        xi = np.zeros((BPAD * N, DM), dtype=np.float32)
        xi[: BPC * N] = x[i * BPC : (i + 1) * BPC].reshape(BPC * N, DM)
        in_maps.append({"x": xi, "tick": tick})

    res = run_bass_kernel_spmd(nc, in_maps, core_ids=list(range(NCORES)))
    LAST_RESULTS = res

    out = np.empty((B, N, N), dtype=np.float32)
    for i in range(NCORES):
        out[i * BPC : (i + 1) * BPC] = res.results[i]["out"].reshape(BPAD, N, N)[:BPC]
    return out
